# revision 22
# baseline (speedup 1.0000x reference)
"""CRF loss kernel for Trainium2 (8 NeuronCores, Bass/Tile).

Math
----
The reference computes, for a single sequence of SEQ=16384 steps over
TAG=1024 tags:

  forward:  fv_{t+1}[j] = logsumexp_i(fv_t[i] + T[j,i]) + feat_t[j]
  score    = logsumexp_j(fv_SEQ[j] + T[stop,j])
  output   = score - gold_score[k]            (gold is a cheap exact term)

In real space with E = exp(T) this is p_{t+1} = exp(feat_t) * (E @ p_t) —
a chain of 16384 matvecs with one fixed positive matrix.  Products of
positive random matrices forget their initial direction extremely fast
(measured ~15x error decay per step), so the chain is split into 1024
chunks of L=16 steps.  Chunk b is evaluated by an independent chain that
starts K=4 steps early (warm-up) from an arbitrary positive vector;
after warm-up its direction equals the true forward direction to working
precision.  The scalar magnitude is recovered by telescoping per-chunk
log-norm ratios, which only needs each chain's vector 1-norm at its
chunk boundary and at its end.

All 1024 chains run in lockstep: 128 chains per core * 8 cores, each
core doing LEN=20 steps.  One step per core is:

  PSUM q[b=128, j'=1024] = sum_i X[i, b] * Mhat[i, j']   (16 accumulating
        128x128-stationary bf16 matmuls, moving = resident Mhat)
  S = q * FE[s]     (DVE mul with preloaded exp(feat) rows, bf16 out)
  X' = S^T          (8 PE transposes into 2 rotating PSUM banks
                     + 8 DVE copies back to SBUF)

The matmul datapath is bf16 (fp32 matmul streams at 1/4 rate on trn2);
fp32 PSUM accumulation keeps per-step log-increment error ~1e-4 nats,
far inside the telescoping stitch's error budget.  All inputs that are
fixed functions of the problem (Mhat = exp(T^T-delta), exp(feats)
arranged per-chain so no partition-shifted loads are needed, u =
exp(T[stop])) are precomputed on the host; the device runs only the
recurrence, so every DMA is a pure prefetch issued at kernel start,
spread over both HWDGE queues and the gpsimd SWDGE queue.  delta=8
keeps values centered (per-step norm growth is ~e^8); drift over 20
steps is a few e-folds so no per-step normalization is needed.
"""

import sys
import numpy as np
import ml_dtypes

for _p in ("/opt/trn_rl_repo",):
    if _p not in sys.path:
        sys.path.insert(0, _p)

from contextlib import ExitStack

from concourse import bacc, tile
from concourse import mybir
from concourse.bass_utils import run_bass_kernel_spmd

F32 = mybir.dt.float32
BF16 = mybir.dt.bfloat16
FP8 = mybir.dt.float8e4
FP8W = mybir.dt.float8e5
BF = ml_dtypes.bfloat16
F8 = mybir.dt.np(mybir.dt.float8e4)
F8W = mybir.dt.np(mybir.dt.float8e5)

SEQ = 16384
TAG = 1024
P = 128            # partitions / chains per core / PE tile edge
NT = TAG // P      # 8 tag tiles
NCORES = 8
L = 16             # chunk length (steps per chunk)
K = 2              # warm-up steps per chain
LEN = L + K        # lockstep steps per core
DELTA = 8.0        # per-step log-growth folded into Mhat
ROWS_PER_CORE = L * P  # 2048

_compiled = None


def _build_kernel():
    nc = bacc.Bacc(
        "TRN2",
        target_bir_lowering=False,
        debug=False,
        num_devices=NCORES,
    )

    mhat_d = nc.declare_dram_parameter("mhat", [P, NT * TAG], FP8W,
                                       isOutput=False)
    ident = nc.declare_dram_parameter("ident", [P, P], BF16, isOutput=False)
    ucol_d = nc.declare_dram_parameter("ucol", [P, NT], FP8W, isOutput=False)
    initx = nc.declare_dram_parameter("initx", [P, TAG], FP8W, isOutput=False)
    fes_d = nc.declare_dram_parameter("fes", [P, LEN * TAG], FP8,
                                      isOutput=False)
    sums = nc.declare_dram_parameter("sums", [16, P], F32, isOutput=True)

    with tile.TileContext(nc) as tc, ExitStack() as ctx:
        const_pool = ctx.enter_context(tc.tile_pool(name="const", bufs=1))
        loop_sb = ctx.enter_context(tc.tile_pool(name="loop_sb", bufs=2))
        qpool = ctx.enter_context(
            tc.tile_pool(name="qpool", bufs=2, space="PSUM"))
        xppool = ctx.enter_context(
            tc.tile_pool(name="xppool", bufs=1, space="PSUM"))

        # state lives in 4 independent quarter-tiles (2 tag-blocks each) so
        # the transpose handoff of quarter i only gates the matmuls reading
        # quarter i — the handoff pipeline overlaps the next MM phase.
        xtq = [loop_sb.tile([P, 2, P], FP8W, tag=f"xt{i}", name=f"xt{i}")
               for i in range(4)]
        for i in range(4):
            eng = (nc.sync, nc.scalar)[i % 2]
            eng.dma_start(xtq[i][:], initx[:, 2 * i * P:(2 * i + 2) * P])

        # Mhat resident in SBUF; 8 it-block DMAs over the two HWDGE queues
        # (DMA triggers cost ~600ns of issuing-engine time, so few and big).
        mh = const_pool.tile([P, NT, TAG], FP8W)
        for c in range(NT):
            eng = (nc.sync, nc.scalar)[c % 2]
            eng.dma_start(mh[:, c, :], mhat_d[:, c * TAG:(c + 1) * TAG])

        idt = const_pool.tile([P, P], BF16)
        nc.sync.dma_start(idt[:], ident[:])
        idt32 = const_pool.tile([P, P], F32)
        nc.scalar.copy(idt32[:], idt[:])
        ucol = const_pool.tile([P, NT], FP8W)
        nc.scalar.dma_start(ucol[:], ucol_d[:])
        recs = const_pool.tile([P, 16], F32)
        nc.gpsimd.memset(recs[:], 0.0)

        # exp(feat) rows (fp8), one window of LEN rows per chain (pre-arranged
        # on host so partition p holds exactly chain p's rows).  Slices for
        # the first 8 steps ride the two HWDGE queues right behind Mhat;
        # the tail goes through the otherwise-idle gpsimd SWDGE queue.
        fes = const_pool.tile([P, LEN * TAG], FP8)
        for c in range(4):
            lo = 2 * c * TAG
            eng = (nc.sync, nc.scalar)[c % 2]
            eng.dma_start(fes[:, lo:lo + 2 * TAG], fes_d[:, lo:lo + 2 * TAG])
        for c in range(3):
            lo = (8 + 4 * c) * TAG
            hi = min(LEN * TAG, lo + 4 * TAG)
            nc.gpsimd.dma_start(fes[:, lo:hi], fes_d[:, lo:hi])

        rec_slot = {K - 1: 0, L - 1: 1, LEN - 1: 2}

        # PE emission interleaves step s's matmuls with step s-1's tail
        # transposes and step s's own leading transposes so the PE queue
        # never drains while the DVE multiply/copy handoff runs.  The
        # transposes land in PSUM banks disjoint from the accumulating
        # q banks, so splicing them inside an accumulation group is safe
        # (skip_group_check).
        def emit_T(jt, stq, xtq, xpab, s):
            xp = xpab[jt // 4][:, (jt % 4) * P:(jt % 4 + 1) * P]
            nc.tensor.matmul(
                xp, lhsT=stq[jt // 2][:, (jt % 2) * P:(jt % 2 + 1) * P],
                rhs=idt[:], is_transpose=True, skip_group_check=True)
            dst = xtq[jt // 2][:, jt % 2, :]
            if jt in (4, 5):
                nc.scalar.copy(dst, xp)
            else:
                nc.vector.tensor_copy(dst, xp)

        def emit_mul(i, qh, stq, s):
            qs = qh[i // 2][:, (i % 2) * 256:(i % 2 + 1) * 256]
            fs = fes[:, s * TAG + i * 256: s * TAG + (i + 1) * 256]
            nc.vector.tensor_mul(stq[i][:], qs, fs)
            if s in rec_slot:
                nc.vector.tensor_reduce(
                    out=recs[:, 4 * rec_slot[s] + i: 4 * rec_slot[s] + i + 1],
                    in_=stq[i][:], op=mybir.AluOpType.add,
                    axis=mybir.AxisListType.X)

        prev = None    # (stq, xtq_new, xpab, s) of previous step
        for s in range(LEN):
            qh = [qpool.tile([P, 512], F32, tag=f"q{h}", name=f"q{h}_{s}")
                  for h in range(2)]
            stq = [loop_sb.tile([P, 256], BF16, tag=f"st{i}",
                                name=f"st{i}_{s}") for i in range(4)]
            nxtq = [loop_sb.tile([P, 2, P], FP8W, tag=f"xt{i}",
                                 name=f"xt{i}_{s}") for i in range(4)]
            xpab = (xppool.tile([P, 512], BF16, tag="xpa", name=f"xpa_{s}"),
                    xppool.tile([P, 512], BF16, tag="xpb", name=f"xpb_{s}"))

            for i in range(2):
                for h in range(2):
                    nc.tensor.matmul(
                        qh[h][:, :], lhsT=xtq[i][:, :, :],
                        rhs=mh[:, 2 * i:2 * i + 2, h * 512:(h + 1) * 512],
                        start=(i == 0), stop=False,
                        perf_mode=mybir.MatmulPerfMode.DoubleRow,
                        skip_group_check=True)
            if prev is not None:
                for jt in (4, 5, 6, 7):
                    emit_T(jt, *prev)
            for i in range(2, 4):
                for h in range(2):
                    nc.tensor.matmul(
                        qh[h][:, :], lhsT=xtq[i][:, :, :],
                        rhs=mh[:, 2 * i:2 * i + 2, h * 512:(h + 1) * 512],
                        start=False, stop=(i == 3),
                        perf_mode=mybir.MatmulPerfMode.DoubleRow,
                        skip_group_check=True)
            for i in range(4):
                emit_mul(i, qh, stq, s)
            for jt in (0, 1, 2, 3):
                emit_T(jt, stq, nxtq, xpab, s)

            prev = (stq, nxtq, xpab, s)
            xtq = nxtq
        for jt in (4, 5, 6, 7):
            emit_T(jt, *prev)

        # ---- dots[b] = sum_j u[j] * X_end[j, b]  (X_end = S_end^T)
        dots_ps = xppool.tile([P, 1], F32, tag="dots", bufs=1)
        for it in range(NT):
            nc.tensor.matmul(
                dots_ps[:], lhsT=xtq[it // 2][:, it % 2, :],
                rhs=ucol[:, it:it + 1], start=(it == 0),
                stop=(it == NT - 1))
        nc.vector.tensor_copy(recs[:, 12:13], dots_ps[:])

        # recs [128, 16] -> transpose on PE -> single contiguous DMA out
        rec_ps = xppool.tile([16, P], F32, tag="rec_ps", bufs=1)
        nc.tensor.transpose(rec_ps[:], recs[:], idt32[:])
        rec_sb = const_pool.tile([16, P], F32)
        nc.vector.tensor_copy(rec_sb[:], rec_ps[:])
        nc.sync.dma_start(sums[:], rec_sb[:])

    nc.compile()
    return nc


def kernel(feats, transitions, tags, start_idx, stop_idx):
    global _compiled
    feats = np.ascontiguousarray(np.asarray(feats, dtype=np.float32))
    T = np.ascontiguousarray(np.asarray(transitions, dtype=np.float32))
    tags_np = np.asarray(tags).astype(np.int64)
    start_i = int(np.asarray(start_idx))
    stop_i = int(np.asarray(stop_idx))

    # ---- gold score entirely on host (cheap, exact)
    tags_ext = np.concatenate([np.array([start_i], dtype=np.int64), tags_np])
    trans_sum = T[tags_ext[1:], tags_ext[:-1]].astype(np.float64).sum()
    counts = np.bincount(tags_ext[1:], minlength=TAG).astype(np.float64)
    emit = counts @ feats[:TAG].astype(np.float64)          # [TAG]
    gold_vec = trans_sum + emit + np.float64(T[stop_i, tags_ext[-1]])

    # ---- fixed input transforms on host
    # Mhat[i, j'] = exp(T[j', i] - DELTA), blocked [128, it*1024 + j']
    Mh = np.exp(T.T.astype(np.float32) - np.float32(DELTA))
    mhat = np.ascontiguousarray(
        Mh.reshape(NT, P, TAG).transpose(1, 0, 2).reshape(P, NT * TAG)
    ).astype(F8W)
    # u[p, jt] = exp(T[stop, jt*128+p])
    ucol = np.ascontiguousarray(
        np.exp(T[stop_i].astype(np.float32)).reshape(NT, P).T).astype(F8W)
    ident = np.eye(P, dtype=np.float32).astype(BF)

    fe_all = np.exp(feats).astype(F8)       # [SEQ, TAG]

    in_maps = []
    for g in range(NCORES):
        # chain b of core g covers global chunk a=128g+b (seq [16a,16a+16)),
        # warming up from seq 16a-K; chain 0 of core 0 starts exactly at 0.
        a0 = 128 * g
        idx = (16 * (a0 + np.arange(P))[:, None] - K
               + np.arange(LEN)[None, :])          # [P, LEN]
        if g == 0:
            idx[0] = np.arange(LEN)
        win = fe_all[idx]                           # [P, LEN, TAG]
        fes = np.ascontiguousarray(win.reshape(P, LEN * TAG))

        x0 = np.ones((TAG, P), np.float32)
        if g == 0:
            x0[:, 0] = 0.0
            x0[start_i, 0] = 1.0
        x0_t = np.ascontiguousarray(
            x0.reshape(NT, P, P).transpose(1, 0, 2).reshape(P, NT * P)
        ).astype(F8W)
        in_maps.append({
            "mhat": mhat, "ucol": ucol, "ident": ident, "initx": x0_t,
            "fes": fes,
        })

    if _compiled is None:
        _compiled = _build_kernel()
    res = run_bass_kernel_spmd(_compiled, in_maps, list(range(NCORES)))
    results = res.results

    # ---- stitch (host: ~2k scalars)
    def rec(slot):
        return np.concatenate(
            [sum(results[g]["sums"][4 * slot + i] for i in range(4))
             for g in range(NCORES)]).astype(np.float64)

    recK = rec(0)      # norm at chunk-start boundary (after warm-up)
    recL = rec(1)      # norm at end of chunk 0 (chain 0 of core 0 only)
    end = rec(2)       # norm at chain end
    d = float(results[NCORES - 1]["sums"][12][P - 1])

    fs = (np.log(d) - np.log(end[TAG - 1])
          + float(np.sum(np.log(end[1:]) - np.log(recK[1:])))
          + np.log(recL[0]) + SEQ * DELTA)
    out = (fs - gold_vec).astype(np.float32)
    return out


# revision 23
# speedup vs baseline: 1.3942x; 1.3942x over previous
"""CRF loss kernel for Trainium2 (8 NeuronCores, Bass/Tile).

Math
----
The reference computes, for a single sequence of SEQ=16384 steps over
TAG=1024 tags:

  forward:  fv_{t+1}[j] = logsumexp_i(fv_t[i] + T[j,i]) + feat_t[j]
  score    = logsumexp_j(fv_SEQ[j] + T[stop,j])
  output   = score - gold_score[k]            (gold is a cheap exact term)

In real space with E = exp(T) this is p_{t+1} = exp(feat_t) * (E @ p_t) —
a chain of 16384 matvecs with one fixed positive matrix.  Products of
positive random matrices forget their initial direction extremely fast
(measured ~15x error decay per step), so the chain is split into 1024
chunks of L=16 steps.  Chunk b is evaluated by an independent chain that
starts K=4 steps early (warm-up) from an arbitrary positive vector;
after warm-up its direction equals the true forward direction to working
precision.  The scalar magnitude is recovered by telescoping per-chunk
log-norm ratios, which only needs each chain's vector 1-norm at its
chunk boundary and at its end.

All 1024 chains run in lockstep: 128 chains per core * 8 cores, each
core doing LEN=20 steps.  One step per core is:

  PSUM q[b=128, j'=1024] = sum_i X[i, b] * Mhat[i, j']   (16 accumulating
        128x128-stationary bf16 matmuls, moving = resident Mhat)
  S = q * FE[s]     (DVE mul with preloaded exp(feat) rows, bf16 out)
  X' = S^T          (8 PE transposes into 2 rotating PSUM banks
                     + 8 DVE copies back to SBUF)

The matmul datapath is bf16 (fp32 matmul streams at 1/4 rate on trn2);
fp32 PSUM accumulation keeps per-step log-increment error ~1e-4 nats,
far inside the telescoping stitch's error budget.  All inputs that are
fixed functions of the problem (Mhat = exp(T^T-delta), exp(feats)
arranged per-chain so no partition-shifted loads are needed, u =
exp(T[stop])) are precomputed on the host; the device runs only the
recurrence, so every DMA is a pure prefetch issued at kernel start,
spread over both HWDGE queues and the gpsimd SWDGE queue.  delta=8
keeps values centered (per-step norm growth is ~e^8); drift over 20
steps is a few e-folds so no per-step normalization is needed.
"""

import sys
import numpy as np
import ml_dtypes

for _p in ("/opt/trn_rl_repo",):
    if _p not in sys.path:
        sys.path.insert(0, _p)

from contextlib import ExitStack

from concourse import bacc, tile
from concourse import mybir
from concourse.bass_utils import run_bass_kernel_spmd

F32 = mybir.dt.float32
BF16 = mybir.dt.bfloat16
FP8 = mybir.dt.float8e4
BF = ml_dtypes.bfloat16
F8 = mybir.dt.np(mybir.dt.float8e4)

SEQ = 16384
TAG = 1024
P = 128            # partitions / chains per core / PE tile edge
NT = TAG // P      # 8 tag tiles
NCORES = 8
L = 16             # chunk length (steps per chunk)
K = 2              # warm-up steps per chain
LEN = L + K        # lockstep steps per core
DELTA = 8.0        # per-step log-growth folded into Mhat
ROWS_PER_CORE = L * P  # 2048

_compiled = None


def _build_kernel():
    nc = bacc.Bacc(
        "TRN2",
        target_bir_lowering=False,
        debug=False,
        num_devices=NCORES,
    )

    mhat_d = nc.declare_dram_parameter("mhat", [P, NT * TAG], BF16,
                                       isOutput=False)
    ident = nc.declare_dram_parameter("ident", [P, P], BF16, isOutput=False)
    ucol_d = nc.declare_dram_parameter("ucol", [P, NT], BF16, isOutput=False)
    initx = nc.declare_dram_parameter("initx", [P, TAG], BF16, isOutput=False)
    fes_d = nc.declare_dram_parameter("fes", [P, LEN * TAG], FP8,
                                      isOutput=False)
    sums = nc.declare_dram_parameter("sums", [16, P], F32, isOutput=True)

    with tile.TileContext(nc) as tc, ExitStack() as ctx:
        const_pool = ctx.enter_context(tc.tile_pool(name="const", bufs=1))
        loop_sb = ctx.enter_context(tc.tile_pool(name="loop_sb", bufs=2))
        qpool = ctx.enter_context(
            tc.tile_pool(name="qpool", bufs=2, space="PSUM"))
        xppool = ctx.enter_context(
            tc.tile_pool(name="xppool", bufs=1, space="PSUM"))

        # state lives in 4 independent quarter-tiles (2 tag-blocks each) so
        # the transpose handoff of quarter i only gates the matmuls reading
        # quarter i — the handoff pipeline overlaps the next MM phase.
        xtq = [loop_sb.tile([P, 2, P], BF16, tag=f"xt{i}", name=f"xt{i}")
               for i in range(4)]
        for i in range(4):
            eng = (nc.sync, nc.scalar)[i % 2]
            eng.dma_start(xtq[i][:], initx[:, 2 * i * P:(2 * i + 2) * P])

        # Mhat resident in SBUF; 8 it-block DMAs over the two HWDGE queues
        # (DMA triggers cost ~600ns of issuing-engine time, so few and big).
        mh = const_pool.tile([P, NT * TAG], BF16)
        for c in range(NT):
            lo = c * TAG
            eng = (nc.sync, nc.scalar)[c % 2]
            eng.dma_start(mh[:, lo:lo + TAG], mhat_d[:, lo:lo + TAG])

        idt = const_pool.tile([P, P], BF16)
        nc.sync.dma_start(idt[:], ident[:])
        idt32 = const_pool.tile([P, P], F32)
        nc.scalar.copy(idt32[:], idt[:])
        ucol = const_pool.tile([P, NT], BF16)
        nc.scalar.dma_start(ucol[:], ucol_d[:])
        recs = const_pool.tile([P, 16], F32)
        nc.gpsimd.memset(recs[:], 0.0)

        # exp(feat) rows (fp8), one window of LEN rows per chain (pre-arranged
        # on host so partition p holds exactly chain p's rows).  Slices for
        # the first 8 steps ride the two HWDGE queues right behind Mhat;
        # the tail goes through the otherwise-idle gpsimd SWDGE queue.
        fes = const_pool.tile([P, LEN * TAG], FP8)
        for c in range(4):
            lo = 2 * c * TAG
            eng = (nc.sync, nc.scalar)[c % 2]
            eng.dma_start(fes[:, lo:lo + 2 * TAG], fes_d[:, lo:lo + 2 * TAG])
        for c in range(3):
            lo = (8 + 4 * c) * TAG
            hi = min(LEN * TAG, lo + 4 * TAG)
            nc.gpsimd.dma_start(fes[:, lo:hi], fes_d[:, lo:hi])

        rec_slot = {K - 1: 0, L - 1: 1, LEN - 1: 2}

        # PE emission interleaves step s's matmuls with step s-1's tail
        # transposes and step s's own leading transposes so the PE queue
        # never drains while the DVE multiply/copy handoff runs.  The
        # transposes land in PSUM banks disjoint from the accumulating
        # q banks, so splicing them inside an accumulation group is safe
        # (skip_group_check).
        def emit_T(jt, stq, xtq, xpab, s):
            xp = xpab[jt // 4][:, (jt % 4) * P:(jt % 4 + 1) * P]
            nc.tensor.matmul(
                xp, lhsT=stq[jt // 2][:, (jt % 2) * P:(jt % 2 + 1) * P],
                rhs=idt[:], is_transpose=True, skip_group_check=True)
            dst = xtq[jt // 2][:, jt % 2, :]
            if jt < 4:
                nc.vector.tensor_copy(dst, xp)
            else:
                nc.scalar.copy(dst, xp)

        def emit_mul(i, qh, stq, s):
            qs = qh[i // 2][:, (i % 2) * 256:(i % 2 + 1) * 256]
            fs = fes[:, s * TAG + i * 256: s * TAG + (i + 1) * 256]
            nc.vector.tensor_mul(stq[i][:], qs, fs)
            if s in rec_slot:
                nc.vector.tensor_reduce(
                    out=recs[:, 4 * rec_slot[s] + i: 4 * rec_slot[s] + i + 1],
                    in_=stq[i][:], op=mybir.AluOpType.add,
                    axis=mybir.AxisListType.X)

        prev = None    # (stq, xtq_new, xpab, s) of previous step
        for s in range(LEN):
            qh = [qpool.tile([P, 512], F32, tag=f"q{h}", name=f"q{h}_{s}")
                  for h in range(2)]
            stq = [loop_sb.tile([P, 256], BF16, tag=f"st{i}",
                                name=f"st{i}_{s}") for i in range(4)]
            nxtq = [loop_sb.tile([P, 2, P], BF16, tag=f"xt{i}",
                                 name=f"xt{i}_{s}") for i in range(4)]
            xpab = (xppool.tile([P, 512], BF16, tag="xpa", name=f"xpa_{s}"),
                    xppool.tile([P, 512], BF16, tag="xpb", name=f"xpb_{s}"))

            for it in range(4):
                nc.tensor.matmul(
                    qh[0][:, :], lhsT=xtq[it // 2][:, it % 2, :],
                    rhs=mh[:, it * TAG: it * TAG + 512],
                    start=(it == 0), stop=False, skip_group_check=True)
            if prev is not None:
                for jt in (4, 5, 6, 7):
                    emit_T(jt, *prev)
            for it in range(4, NT):
                nc.tensor.matmul(
                    qh[0][:, :], lhsT=xtq[it // 2][:, it % 2, :],
                    rhs=mh[:, it * TAG: it * TAG + 512],
                    start=False, stop=(it == NT - 1), skip_group_check=True)
            emit_mul(0, qh, stq, s)
            emit_mul(1, qh, stq, s)
            for it in range(4):
                nc.tensor.matmul(
                    qh[1][:, :], lhsT=xtq[it // 2][:, it % 2, :],
                    rhs=mh[:, it * TAG + 512: (it + 1) * TAG],
                    start=(it == 0), stop=False, skip_group_check=True)
            for jt in (0, 1, 2, 3):
                emit_T(jt, stq, nxtq, xpab, s)
            for it in range(4, NT):
                nc.tensor.matmul(
                    qh[1][:, :], lhsT=xtq[it // 2][:, it % 2, :],
                    rhs=mh[:, it * TAG + 512: (it + 1) * TAG],
                    start=False, stop=(it == NT - 1), skip_group_check=True)
            emit_mul(2, qh, stq, s)
            emit_mul(3, qh, stq, s)

            prev = (stq, nxtq, xpab, s)
            xtq = nxtq
        for jt in (4, 5, 6, 7):
            emit_T(jt, *prev)

        # ---- dots[b] = sum_j u[j] * X_end[j, b]  (X_end = S_end^T)
        dots_ps = xppool.tile([P, 1], F32, tag="dots", bufs=1)
        for it in range(NT):
            nc.tensor.matmul(
                dots_ps[:], lhsT=xtq[it // 2][:, it % 2, :],
                rhs=ucol[:, it:it + 1], start=(it == 0),
                stop=(it == NT - 1))
        nc.vector.tensor_copy(recs[:, 12:13], dots_ps[:])

        # recs [128, 16] -> transpose on PE -> single contiguous DMA out
        rec_ps = xppool.tile([16, P], F32, tag="rec_ps", bufs=1)
        nc.tensor.transpose(rec_ps[:], recs[:], idt32[:])
        rec_sb = const_pool.tile([16, P], F32)
        nc.vector.tensor_copy(rec_sb[:], rec_ps[:])
        nc.sync.dma_start(sums[:], rec_sb[:])

    nc.compile()
    return nc


def kernel(feats, transitions, tags, start_idx, stop_idx):
    global _compiled
    feats = np.ascontiguousarray(np.asarray(feats, dtype=np.float32))
    T = np.ascontiguousarray(np.asarray(transitions, dtype=np.float32))
    tags_np = np.asarray(tags).astype(np.int64)
    start_i = int(np.asarray(start_idx))
    stop_i = int(np.asarray(stop_idx))

    # ---- gold score entirely on host (cheap, exact)
    tags_ext = np.concatenate([np.array([start_i], dtype=np.int64), tags_np])
    trans_sum = T[tags_ext[1:], tags_ext[:-1]].astype(np.float64).sum()
    counts = np.bincount(tags_ext[1:], minlength=TAG).astype(np.float64)
    emit = counts @ feats[:TAG].astype(np.float64)          # [TAG]
    gold_vec = trans_sum + emit + np.float64(T[stop_i, tags_ext[-1]])

    # ---- fixed input transforms on host
    # Mhat[i, j'] = exp(T[j', i] - DELTA), blocked [128, it*1024 + j']
    Mh = np.exp(T.T.astype(np.float32) - np.float32(DELTA))
    mhat = np.ascontiguousarray(
        Mh.reshape(NT, P, TAG).transpose(1, 0, 2).reshape(P, NT * TAG)
    ).astype(BF)
    # u[p, jt] = exp(T[stop, jt*128+p])
    ucol = np.ascontiguousarray(
        np.exp(T[stop_i].astype(np.float32)).reshape(NT, P).T).astype(BF)
    ident = np.eye(P, dtype=np.float32).astype(BF)

    fe_all = np.exp(feats).astype(F8)       # [SEQ, TAG]

    in_maps = []
    for g in range(NCORES):
        # chain b of core g covers global chunk a=128g+b (seq [16a,16a+16)),
        # warming up from seq 16a-K; chain 0 of core 0 starts exactly at 0.
        a0 = 128 * g
        idx = (16 * (a0 + np.arange(P))[:, None] - K
               + np.arange(LEN)[None, :])          # [P, LEN]
        if g == 0:
            idx[0] = np.arange(LEN)
        win = fe_all[idx]                           # [P, LEN, TAG]
        fes = np.ascontiguousarray(win.reshape(P, LEN * TAG))

        x0 = np.ones((TAG, P), np.float32)
        if g == 0:
            x0[:, 0] = 0.0
            x0[start_i, 0] = 1.0
        x0_t = np.ascontiguousarray(
            x0.reshape(NT, P, P).transpose(1, 0, 2).reshape(P, NT * P)
        ).astype(BF)
        in_maps.append({
            "mhat": mhat, "ucol": ucol, "ident": ident, "initx": x0_t,
            "fes": fes,
        })

    if _compiled is None:
        _compiled = _build_kernel()
    res = run_bass_kernel_spmd(_compiled, in_maps, list(range(NCORES)))
    results = res.results

    # ---- stitch (host: ~2k scalars)
    def rec(slot):
        return np.concatenate(
            [sum(results[g]["sums"][4 * slot + i] for i in range(4))
             for g in range(NCORES)]).astype(np.float64)

    recK = rec(0)      # norm at chunk-start boundary (after warm-up)
    recL = rec(1)      # norm at end of chunk 0 (chain 0 of core 0 only)
    end = rec(2)       # norm at chain end
    d = float(results[NCORES - 1]["sums"][12][P - 1])

    fs = (np.log(d) - np.log(end[TAG - 1])
          + float(np.sum(np.log(end[1:]) - np.log(recK[1:])))
          + np.log(recL[0]) + SEQ * DELTA)
    out = (fs - gold_vec).astype(np.float32)
    return out


# revision 24
# speedup vs baseline: 1.4044x; 1.0073x over previous
"""CRF loss kernel for Trainium2 (8 NeuronCores, Bass/Tile).

Math
----
The reference computes, for a single sequence of SEQ=16384 steps over
TAG=1024 tags:

  forward:  fv_{t+1}[j] = logsumexp_i(fv_t[i] + T[j,i]) + feat_t[j]
  score    = logsumexp_j(fv_SEQ[j] + T[stop,j])
  output   = score - gold_score[k]            (gold is a cheap exact term)

In real space with E = exp(T) this is p_{t+1} = exp(feat_t) * (E @ p_t) —
a chain of 16384 matvecs with one fixed positive matrix.  Products of
positive random matrices forget their initial direction extremely fast
(measured ~15x error decay per step, far below working precision after
2 steps), so the chain is split into 1024 chunks of L=16 steps.  Chunk b
is evaluated by an independent chain that starts K=2 steps early
(warm-up) from an arbitrary positive vector; after warm-up its direction
equals the true forward direction to working precision.  The scalar
magnitude is recovered by telescoping per-chunk log-norm ratios, which
only needs each chain's vector 1-norm at its chunk boundary and at its
end.

All 1024 chains run in lockstep: 128 chains per core * 8 cores, each
core doing LEN=18 steps.  One step per core is:

  PSUM q[b=128, j'=1024] = sum_i X[i, b] * Mhat[i, j']   (16 accumulating
        128x128-stationary bf16 matmuls, moving = resident Mhat)
  S = q * FE[s]     (4 DVE quarter-multiplies with preloaded fp8
                     exp(feat) rows, bf16 out)
  X' = S^T          (8 PE transposes into 2 rotating PSUM banks +
                     copies back to SBUF split over DVE and ScalarE)

The matmul datapath is bf16 (fp32 matmul streams at 1/4 rate on trn2;
fp8 DoubleRow measured slower here because the stationary operand
changes every matmul and DoubleRow disables fast-weight-load).  fp32
PSUM accumulation keeps per-step log-increment error ~1e-4 nats, far
inside the telescoping stitch's error budget.

Scheduling: state/q/S live in independent quarter-tiles so dependencies
are tile-precise, and the PE emission interleaves step s's accumulation
groups with step s-1's tail transposes and step s's leading transposes
(disjoint PSUM banks, skip_group_check) — the PE queue never drains, so
the HAM clock-gate stays at full rate.  All inputs that are fixed
functions of the problem (Mhat = exp(T^T-delta) in bf16, exp(feats) in
fp8e4m3 arranged per-chain so no partition-shifted loads are needed,
u = exp(T[stop]), the gold score) are precomputed on the host; every
DMA is a pure prefetch spread over both HWDGE queues and the gpsimd
SWDGE queue, and the 13 result scalars per chain leave through a single
on-device transpose + one contiguous DMA.  delta=8 keeps values
centered (per-step norm growth is ~e^8); drift over 18 steps is a few
e-folds so no per-step normalization is needed.
"""

import sys
import numpy as np
import ml_dtypes

for _p in ("/opt/trn_rl_repo",):
    if _p not in sys.path:
        sys.path.insert(0, _p)

from contextlib import ExitStack

from concourse import bacc, tile
from concourse import mybir
from concourse.bass_utils import run_bass_kernel_spmd

F32 = mybir.dt.float32
BF16 = mybir.dt.bfloat16
FP8 = mybir.dt.float8e4
BF = ml_dtypes.bfloat16
F8 = mybir.dt.np(mybir.dt.float8e4)

SEQ = 16384
TAG = 1024
P = 128            # partitions / chains per core / PE tile edge
NT = TAG // P      # 8 tag tiles
NCORES = 8
L = 16             # chunk length (steps per chunk)
K = 2              # warm-up steps per chain
LEN = L + K        # lockstep steps per core
DELTA = 8.0        # per-step log-growth folded into Mhat
ROWS_PER_CORE = L * P  # 2048

_compiled = None


def _build_kernel():
    nc = bacc.Bacc(
        "TRN2",
        target_bir_lowering=False,
        debug=False,
        num_devices=NCORES,
    )

    mhat_d = nc.declare_dram_parameter("mhat", [P, NT * TAG], BF16,
                                       isOutput=False)
    ident = nc.declare_dram_parameter("ident", [P, P], BF16, isOutput=False)
    ucol_d = nc.declare_dram_parameter("ucol", [P, NT], BF16, isOutput=False)
    initx = nc.declare_dram_parameter("initx", [P, TAG], BF16, isOutput=False)
    fes_d = nc.declare_dram_parameter("fes", [P, LEN * TAG], FP8,
                                      isOutput=False)
    sums = nc.declare_dram_parameter("sums", [16, P], F32, isOutput=True)

    with tile.TileContext(nc) as tc, ExitStack() as ctx:
        const_pool = ctx.enter_context(tc.tile_pool(name="const", bufs=1))
        loop_sb = ctx.enter_context(tc.tile_pool(name="loop_sb", bufs=2))
        qpool = ctx.enter_context(
            tc.tile_pool(name="qpool", bufs=2, space="PSUM"))
        xppool = ctx.enter_context(
            tc.tile_pool(name="xppool", bufs=1, space="PSUM"))

        # state lives in 4 independent quarter-tiles (2 tag-blocks each) so
        # the transpose handoff of quarter i only gates the matmuls reading
        # quarter i — the handoff pipeline overlaps the next MM phase.
        xtq = [loop_sb.tile([P, 2, P], BF16, tag=f"xt{i}", name=f"xt{i}")
               for i in range(4)]
        for i in range(4):
            eng = (nc.sync, nc.scalar)[i % 2]
            eng.dma_start(xtq[i][:], initx[:, 2 * i * P:(2 * i + 2) * P])

        # Mhat resident in SBUF; 8 it-block DMAs over the two HWDGE queues
        # (DMA triggers cost ~600ns of issuing-engine time, so few and big).
        mh = const_pool.tile([P, NT * TAG], BF16)
        for c in range(NT):
            lo = c * TAG
            eng = (nc.sync, nc.scalar)[c % 2]
            eng.dma_start(mh[:, lo:lo + TAG], mhat_d[:, lo:lo + TAG])

        idt = const_pool.tile([P, P], BF16)
        nc.sync.dma_start(idt[:], ident[:])
        idt32 = const_pool.tile([P, P], F32)
        nc.scalar.copy(idt32[:], idt[:])
        ucol = const_pool.tile([P, NT], BF16)
        nc.scalar.dma_start(ucol[:], ucol_d[:])
        recs = const_pool.tile([P, 16], F32)
        nc.gpsimd.memset(recs[:], 0.0)

        # exp(feat) rows (fp8), one window of LEN rows per chain (pre-arranged
        # on host so partition p holds exactly chain p's rows).  Slices for
        # the first 8 steps ride the two HWDGE queues right behind Mhat;
        # the tail goes through the otherwise-idle gpsimd SWDGE queue.
        fes = const_pool.tile([P, LEN * TAG], FP8)
        for c in range(4):
            lo = 2 * c * TAG
            eng = (nc.sync, nc.scalar)[c % 2]
            eng.dma_start(fes[:, lo:lo + 2 * TAG], fes_d[:, lo:lo + 2 * TAG])
        for c in range(3):
            lo = (8 + 4 * c) * TAG
            hi = min(LEN * TAG, lo + 4 * TAG)
            nc.gpsimd.dma_start(fes[:, lo:hi], fes_d[:, lo:hi])

        rec_slot = {K - 1: 0, L - 1: 1, LEN - 1: 2}

        # PE emission interleaves step s's matmuls with step s-1's tail
        # transposes and step s's own leading transposes so the PE queue
        # never drains while the DVE multiply/copy handoff runs.  The
        # transposes land in PSUM banks disjoint from the accumulating
        # q banks, so splicing them inside an accumulation group is safe
        # (skip_group_check).
        def emit_T(jt, stq, xtq, xpab, s):
            xp = xpab[jt // 4][:, (jt % 4) * P:(jt % 4 + 1) * P]
            nc.tensor.matmul(
                xp, lhsT=stq[jt // 2][:, (jt % 2) * P:(jt % 2 + 1) * P],
                rhs=idt[:], is_transpose=True, skip_group_check=True)
            dst = xtq[jt // 2][:, jt % 2, :]
            if jt < 4:
                nc.vector.tensor_copy(dst, xp)
            else:
                nc.scalar.copy(dst, xp)

        def emit_mul(i, qh, stq, s):
            qs = qh[i // 2][:, (i % 2) * 256:(i % 2 + 1) * 256]
            fs = fes[:, s * TAG + i * 256: s * TAG + (i + 1) * 256]
            nc.vector.tensor_mul(stq[i][:], qs, fs)
            if s in rec_slot:
                nc.vector.tensor_reduce(
                    out=recs[:, 4 * rec_slot[s] + i: 4 * rec_slot[s] + i + 1],
                    in_=stq[i][:], op=mybir.AluOpType.add,
                    axis=mybir.AxisListType.X)

        prev = None    # (stq, xtq_new, xpab, s) of previous step
        for s in range(LEN):
            qh = [qpool.tile([P, 512], F32, tag=f"q{h}", name=f"q{h}_{s}")
                  for h in range(2)]
            stq = [loop_sb.tile([P, 256], BF16, tag=f"st{i}",
                                name=f"st{i}_{s}") for i in range(4)]
            nxtq = [loop_sb.tile([P, 2, P], BF16, tag=f"xt{i}",
                                 name=f"xt{i}_{s}") for i in range(4)]
            xpab = (xppool.tile([P, 512], BF16, tag="xpa", name=f"xpa_{s}"),
                    xppool.tile([P, 512], BF16, tag="xpb", name=f"xpb_{s}"))

            for it in range(4):
                nc.tensor.matmul(
                    qh[0][:, :], lhsT=xtq[it // 2][:, it % 2, :],
                    rhs=mh[:, it * TAG: it * TAG + 512],
                    start=(it == 0), stop=False, skip_group_check=True)
            if prev is not None:
                for jt in (4, 5, 6, 7):
                    emit_T(jt, *prev)
            for it in range(4, NT):
                nc.tensor.matmul(
                    qh[0][:, :], lhsT=xtq[it // 2][:, it % 2, :],
                    rhs=mh[:, it * TAG: it * TAG + 512],
                    start=False, stop=(it == NT - 1), skip_group_check=True)
            emit_mul(0, qh, stq, s)
            emit_mul(1, qh, stq, s)
            for it in range(4):
                nc.tensor.matmul(
                    qh[1][:, :], lhsT=xtq[it // 2][:, it % 2, :],
                    rhs=mh[:, it * TAG + 512: (it + 1) * TAG],
                    start=(it == 0), stop=False, skip_group_check=True)
            for jt in (0, 1, 2, 3):
                emit_T(jt, stq, nxtq, xpab, s)
            for it in range(4, NT):
                nc.tensor.matmul(
                    qh[1][:, :], lhsT=xtq[it // 2][:, it % 2, :],
                    rhs=mh[:, it * TAG + 512: (it + 1) * TAG],
                    start=False, stop=(it == NT - 1), skip_group_check=True)
            emit_mul(2, qh, stq, s)
            emit_mul(3, qh, stq, s)

            prev = (stq, nxtq, xpab, s)
            xtq = nxtq
        for jt in (4, 5, 6, 7):
            emit_T(jt, *prev)

        # ---- dots[b] = sum_j u[j] * X_end[j, b]  (X_end = S_end^T)
        dots_ps = xppool.tile([P, 1], F32, tag="dots", bufs=1)
        for it in range(NT):
            nc.tensor.matmul(
                dots_ps[:], lhsT=xtq[it // 2][:, it % 2, :],
                rhs=ucol[:, it:it + 1], start=(it == 0),
                stop=(it == NT - 1))
        nc.vector.tensor_copy(recs[:, 12:13], dots_ps[:])

        # recs [128, 16] -> transpose on PE -> single contiguous DMA out
        rec_ps = xppool.tile([16, P], F32, tag="rec_ps", bufs=1)
        nc.tensor.transpose(rec_ps[:], recs[:], idt32[:])
        rec_sb = const_pool.tile([16, P], F32)
        nc.vector.tensor_copy(rec_sb[:], rec_ps[:])
        nc.sync.dma_start(sums[:], rec_sb[:])

    nc.compile()
    return nc


def kernel(feats, transitions, tags, start_idx, stop_idx):
    global _compiled
    feats = np.ascontiguousarray(np.asarray(feats, dtype=np.float32))
    T = np.ascontiguousarray(np.asarray(transitions, dtype=np.float32))
    tags_np = np.asarray(tags).astype(np.int64)
    start_i = int(np.asarray(start_idx))
    stop_i = int(np.asarray(stop_idx))

    # ---- gold score entirely on host (cheap, exact)
    tags_ext = np.concatenate([np.array([start_i], dtype=np.int64), tags_np])
    trans_sum = T[tags_ext[1:], tags_ext[:-1]].astype(np.float64).sum()
    counts = np.bincount(tags_ext[1:], minlength=TAG).astype(np.float64)
    emit = counts @ feats[:TAG].astype(np.float64)          # [TAG]
    gold_vec = trans_sum + emit + np.float64(T[stop_i, tags_ext[-1]])

    # ---- fixed input transforms on host
    # Mhat[i, j'] = exp(T[j', i] - DELTA), blocked [128, it*1024 + j']
    Mh = np.exp(T.T.astype(np.float32) - np.float32(DELTA))
    mhat = np.ascontiguousarray(
        Mh.reshape(NT, P, TAG).transpose(1, 0, 2).reshape(P, NT * TAG)
    ).astype(BF)
    # u[p, jt] = exp(T[stop, jt*128+p])
    ucol = np.ascontiguousarray(
        np.exp(T[stop_i].astype(np.float32)).reshape(NT, P).T).astype(BF)
    ident = np.eye(P, dtype=np.float32).astype(BF)

    fe_all = np.exp(feats).astype(F8)       # [SEQ, TAG]

    in_maps = []
    for g in range(NCORES):
        # chain b of core g covers global chunk a=128g+b (seq [16a,16a+16)),
        # warming up from seq 16a-K; chain 0 of core 0 starts exactly at 0.
        a0 = 128 * g
        idx = (16 * (a0 + np.arange(P))[:, None] - K
               + np.arange(LEN)[None, :])          # [P, LEN]
        if g == 0:
            idx[0] = np.arange(LEN)
        win = fe_all[idx]                           # [P, LEN, TAG]
        fes = np.ascontiguousarray(win.reshape(P, LEN * TAG))

        x0 = np.ones((TAG, P), np.float32)
        if g == 0:
            x0[:, 0] = 0.0
            x0[start_i, 0] = 1.0
        x0_t = np.ascontiguousarray(
            x0.reshape(NT, P, P).transpose(1, 0, 2).reshape(P, NT * P)
        ).astype(BF)
        in_maps.append({
            "mhat": mhat, "ucol": ucol, "ident": ident, "initx": x0_t,
            "fes": fes,
        })

    if _compiled is None:
        _compiled = _build_kernel()
    res = run_bass_kernel_spmd(_compiled, in_maps, list(range(NCORES)))
    results = res.results

    # ---- stitch (host: ~2k scalars)
    def rec(slot):
        return np.concatenate(
            [sum(results[g]["sums"][4 * slot + i] for i in range(4))
             for g in range(NCORES)]).astype(np.float64)

    recK = rec(0)      # norm at chunk-start boundary (after warm-up)
    recL = rec(1)      # norm at end of chunk 0 (chain 0 of core 0 only)
    end = rec(2)       # norm at chain end
    d = float(results[NCORES - 1]["sums"][12][P - 1])

    fs = (np.log(d) - np.log(end[TAG - 1])
          + float(np.sum(np.log(end[1:]) - np.log(recK[1:])))
          + np.log(recL[0]) + SEQ * DELTA)
    out = (fs - gold_vec).astype(np.float32)
    return out


# revision 25
# speedup vs baseline: 1.4905x; 1.0613x over previous
"""CRF loss kernel for Trainium2 (8 NeuronCores, Bass/Tile).

Math
----
The reference computes, for a single sequence of SEQ=16384 steps over
TAG=1024 tags:

  forward:  fv_{t+1}[j] = logsumexp_i(fv_t[i] + T[j,i]) + feat_t[j]
  score    = logsumexp_j(fv_SEQ[j] + T[stop,j])
  output   = score - gold_score[k]            (gold is a cheap exact term)

In real space with E = exp(T) this is p_{t+1} = exp(feat_t) * (E @ p_t) —
a chain of 16384 matvecs with one fixed positive matrix.  Products of
positive random matrices forget their initial direction extremely fast
(measured ~15x error decay per step, far below working precision after
2 steps), so the chain is split into 1024 chunks of L=16 steps.  Chunk b
is evaluated by an independent chain that starts K=2 steps early
(warm-up) from an arbitrary positive vector; after warm-up its direction
equals the true forward direction to working precision.  The scalar
magnitude is recovered by telescoping per-chunk log-norm ratios, which
only needs each chain's vector 1-norm at its chunk boundary and at its
end.

All 1024 chains run in lockstep: 128 chains per core * 8 cores, each
core doing LEN=18 steps.  One step per core is:

  PSUM q[b=128, j'=1024] = sum_i X[i, b] * Mhat[i, j']   (16 accumulating
        128x128-stationary bf16 matmuls, moving = resident Mhat)
  S = q * FE[s]     (4 DVE quarter-multiplies with preloaded fp8
                     exp(feat) rows, bf16 out)
  X' = S^T          (8 PE transposes into 2 rotating PSUM banks +
                     copies back to SBUF split over DVE and ScalarE)

The matmul datapath is bf16 (fp32 matmul streams at 1/4 rate on trn2;
fp8 DoubleRow measured slower here because the stationary operand
changes every matmul and DoubleRow disables fast-weight-load).  fp32
PSUM accumulation keeps per-step log-increment error ~1e-4 nats, far
inside the telescoping stitch's error budget.

Scheduling: state/q/S live in independent quarter-tiles so dependencies
are tile-precise, and the PE emission interleaves step s's accumulation
groups with step s-1's tail transposes and step s's leading transposes
(disjoint PSUM banks, skip_group_check) — the PE queue never drains, so
the HAM clock-gate stays at full rate.  All inputs that are fixed
functions of the problem (Mhat = exp(T^T-delta) in bf16, exp(feats) in
fp8e4m3 arranged per-chain so no partition-shifted loads are needed,
u = exp(T[stop]), the gold score) are precomputed on the host; every
DMA is a pure prefetch spread over both HWDGE queues and the gpsimd
SWDGE queue, and the 13 result scalars per chain leave through a single
on-device transpose + one contiguous DMA.  delta=8 keeps values
centered (per-step norm growth is ~e^8); drift over 18 steps is a few
e-folds so no per-step normalization is needed.
"""

import sys
import numpy as np
import ml_dtypes

for _p in ("/opt/trn_rl_repo",):
    if _p not in sys.path:
        sys.path.insert(0, _p)

from contextlib import ExitStack

from concourse import bacc, tile
from concourse import mybir
from concourse.bass_utils import run_bass_kernel_spmd

F32 = mybir.dt.float32
BF16 = mybir.dt.bfloat16
FP8 = mybir.dt.float8e4
BF = ml_dtypes.bfloat16
F8 = mybir.dt.np(mybir.dt.float8e4)

SEQ = 16384
TAG = 1024
P = 128            # partitions / chains per core / PE tile edge
NT = TAG // P      # 8 tag tiles
NCORES = 8
L = 16             # chunk length (steps per chunk)
K = 1              # warm-up steps per chain
LEN = L + K        # lockstep steps per core
DELTA = 8.0        # per-step log-growth folded into Mhat
ROWS_PER_CORE = L * P  # 2048

_compiled = None


def _build_kernel():
    nc = bacc.Bacc(
        "TRN2",
        target_bir_lowering=False,
        debug=False,
        num_devices=NCORES,
    )

    mhat_d = nc.declare_dram_parameter("mhat", [P, NT * TAG], BF16,
                                       isOutput=False)
    ident = nc.declare_dram_parameter("ident", [P, P], BF16, isOutput=False)
    ucol_d = nc.declare_dram_parameter("ucol", [P, NT], BF16, isOutput=False)
    initx = nc.declare_dram_parameter("initx", [P, TAG], BF16, isOutput=False)
    fes_d = nc.declare_dram_parameter("fes", [P, LEN * TAG], FP8,
                                      isOutput=False)
    sums = nc.declare_dram_parameter("sums", [16, P], F32, isOutput=True)

    with tile.TileContext(nc) as tc, ExitStack() as ctx:
        const_pool = ctx.enter_context(tc.tile_pool(name="const", bufs=1))
        loop_sb = ctx.enter_context(tc.tile_pool(name="loop_sb", bufs=2))
        qpool = ctx.enter_context(
            tc.tile_pool(name="qpool", bufs=2, space="PSUM"))
        xppool = ctx.enter_context(
            tc.tile_pool(name="xppool", bufs=1, space="PSUM"))

        # state lives in 4 independent quarter-tiles (2 tag-blocks each) so
        # the transpose handoff of quarter i only gates the matmuls reading
        # quarter i — the handoff pipeline overlaps the next MM phase.
        xtq = [loop_sb.tile([P, 2, P], BF16, tag=f"xt{i}", name=f"xt{i}")
               for i in range(4)]
        for i in range(4):
            eng = (nc.sync, nc.scalar)[i % 2]
            eng.dma_start(xtq[i][:], initx[:, 2 * i * P:(2 * i + 2) * P])

        # Mhat resident in SBUF; 8 it-block DMAs over the two HWDGE queues
        # (DMA triggers cost ~600ns of issuing-engine time, so few and big).
        mh = const_pool.tile([P, NT * TAG], BF16)
        for c in range(NT):
            lo = c * TAG
            eng = (nc.sync, nc.scalar)[c % 2]
            eng.dma_start(mh[:, lo:lo + TAG], mhat_d[:, lo:lo + TAG])

        idt = const_pool.tile([P, P], BF16)
        nc.sync.dma_start(idt[:], ident[:])
        idt32 = const_pool.tile([P, P], F32)
        nc.scalar.copy(idt32[:], idt[:])
        ucol = const_pool.tile([P, NT], BF16)
        nc.scalar.dma_start(ucol[:], ucol_d[:])
        recs = const_pool.tile([P, 16], F32)
        nc.gpsimd.memset(recs[:], 0.0)

        # exp(feat) rows (fp8), one window of LEN rows per chain (pre-arranged
        # on host so partition p holds exactly chain p's rows).  Slices for
        # the first 8 steps ride the two HWDGE queues right behind Mhat;
        # the tail goes through the otherwise-idle gpsimd SWDGE queue.
        fes = const_pool.tile([P, LEN * TAG], FP8)
        for c in range(4):
            lo = 2 * c * TAG
            eng = (nc.sync, nc.scalar)[c % 2]
            eng.dma_start(fes[:, lo:lo + 2 * TAG], fes_d[:, lo:lo + 2 * TAG])
        for c in range(3):
            lo = (8 + 4 * c) * TAG
            hi = min(LEN * TAG, lo + 4 * TAG)
            nc.gpsimd.dma_start(fes[:, lo:hi], fes_d[:, lo:hi])

        rec_slot = {K - 1: 0, L - 1: 1, LEN - 1: 2}

        # PE emission interleaves step s's matmuls with step s-1's tail
        # transposes and step s's own leading transposes so the PE queue
        # never drains while the DVE multiply/copy handoff runs.  The
        # transposes land in PSUM banks disjoint from the accumulating
        # q banks, so splicing them inside an accumulation group is safe
        # (skip_group_check).
        def emit_T(jt, stq, xtq, xpab, s):
            xp = xpab[jt // 4][:, (jt % 4) * P:(jt % 4 + 1) * P]
            nc.tensor.matmul(
                xp, lhsT=stq[jt // 2][:, (jt % 2) * P:(jt % 2 + 1) * P],
                rhs=idt[:], is_transpose=True, skip_group_check=True)
            dst = xtq[jt // 2][:, jt % 2, :]
            if jt < 4:
                nc.vector.tensor_copy(dst, xp)
            else:
                nc.scalar.copy(dst, xp)

        def emit_mul(i, qh, stq, s):
            qs = qh[i // 2][:, (i % 2) * 256:(i % 2 + 1) * 256]
            fs = fes[:, s * TAG + i * 256: s * TAG + (i + 1) * 256]
            nc.vector.tensor_mul(stq[i][:], qs, fs)
            if s in rec_slot:
                nc.vector.tensor_reduce(
                    out=recs[:, 4 * rec_slot[s] + i: 4 * rec_slot[s] + i + 1],
                    in_=stq[i][:], op=mybir.AluOpType.add,
                    axis=mybir.AxisListType.X)

        prev = None    # (stq, xtq_new, xpab, s) of previous step
        for s in range(LEN):
            qh = [qpool.tile([P, 512], F32, tag=f"q{h}", name=f"q{h}_{s}")
                  for h in range(2)]
            stq = [loop_sb.tile([P, 256], BF16, tag=f"st{i}",
                                name=f"st{i}_{s}") for i in range(4)]
            nxtq = [loop_sb.tile([P, 2, P], BF16, tag=f"xt{i}",
                                 name=f"xt{i}_{s}") for i in range(4)]
            xpab = (xppool.tile([P, 512], BF16, tag="xpa", name=f"xpa_{s}"),
                    xppool.tile([P, 512], BF16, tag="xpb", name=f"xpb_{s}"))

            for it in range(4):
                nc.tensor.matmul(
                    qh[0][:, :], lhsT=xtq[it // 2][:, it % 2, :],
                    rhs=mh[:, it * TAG: it * TAG + 512],
                    start=(it == 0), stop=False, skip_group_check=True)
            if prev is not None:
                for jt in (4, 5, 6, 7):
                    emit_T(jt, *prev)
            for it in range(4, NT):
                nc.tensor.matmul(
                    qh[0][:, :], lhsT=xtq[it // 2][:, it % 2, :],
                    rhs=mh[:, it * TAG: it * TAG + 512],
                    start=False, stop=(it == NT - 1), skip_group_check=True)
            emit_mul(0, qh, stq, s)
            emit_mul(1, qh, stq, s)
            for it in range(4):
                nc.tensor.matmul(
                    qh[1][:, :], lhsT=xtq[it // 2][:, it % 2, :],
                    rhs=mh[:, it * TAG + 512: (it + 1) * TAG],
                    start=(it == 0), stop=False, skip_group_check=True)
            for jt in (0, 1, 2, 3):
                emit_T(jt, stq, nxtq, xpab, s)
            for it in range(4, NT):
                nc.tensor.matmul(
                    qh[1][:, :], lhsT=xtq[it // 2][:, it % 2, :],
                    rhs=mh[:, it * TAG + 512: (it + 1) * TAG],
                    start=False, stop=(it == NT - 1), skip_group_check=True)
            emit_mul(2, qh, stq, s)
            emit_mul(3, qh, stq, s)

            prev = (stq, nxtq, xpab, s)
            xtq = nxtq
        for jt in (4, 5, 6, 7):
            emit_T(jt, *prev)

        # ---- dots[b] = sum_j u[j] * X_end[j, b]  (X_end = S_end^T)
        dots_ps = xppool.tile([P, 1], F32, tag="dots", bufs=1)
        for it in range(NT):
            nc.tensor.matmul(
                dots_ps[:], lhsT=xtq[it // 2][:, it % 2, :],
                rhs=ucol[:, it:it + 1], start=(it == 0),
                stop=(it == NT - 1))
        nc.vector.tensor_copy(recs[:, 12:13], dots_ps[:])

        # recs [128, 16] -> transpose on PE -> single contiguous DMA out
        rec_ps = xppool.tile([16, P], F32, tag="rec_ps", bufs=1)
        nc.tensor.transpose(rec_ps[:], recs[:], idt32[:])
        rec_sb = const_pool.tile([16, P], F32)
        nc.vector.tensor_copy(rec_sb[:], rec_ps[:])
        nc.sync.dma_start(sums[:], rec_sb[:])

    nc.compile()
    return nc


def kernel(feats, transitions, tags, start_idx, stop_idx):
    global _compiled
    feats = np.ascontiguousarray(np.asarray(feats, dtype=np.float32))
    T = np.ascontiguousarray(np.asarray(transitions, dtype=np.float32))
    tags_np = np.asarray(tags).astype(np.int64)
    start_i = int(np.asarray(start_idx))
    stop_i = int(np.asarray(stop_idx))

    # ---- gold score entirely on host (cheap, exact)
    tags_ext = np.concatenate([np.array([start_i], dtype=np.int64), tags_np])
    trans_sum = T[tags_ext[1:], tags_ext[:-1]].astype(np.float64).sum()
    counts = np.bincount(tags_ext[1:], minlength=TAG).astype(np.float64)
    emit = counts @ feats[:TAG].astype(np.float64)          # [TAG]
    gold_vec = trans_sum + emit + np.float64(T[stop_i, tags_ext[-1]])

    # ---- fixed input transforms on host
    # Mhat[i, j'] = exp(T[j', i] - DELTA), blocked [128, it*1024 + j']
    Mh = np.exp(T.T.astype(np.float32) - np.float32(DELTA))
    mhat = np.ascontiguousarray(
        Mh.reshape(NT, P, TAG).transpose(1, 0, 2).reshape(P, NT * TAG)
    ).astype(BF)
    # u[p, jt] = exp(T[stop, jt*128+p])
    ucol = np.ascontiguousarray(
        np.exp(T[stop_i].astype(np.float32)).reshape(NT, P).T).astype(BF)
    ident = np.eye(P, dtype=np.float32).astype(BF)

    fe_all = np.exp(feats).astype(F8)       # [SEQ, TAG]

    in_maps = []
    for g in range(NCORES):
        # chain b of core g covers global chunk a=128g+b (seq [16a,16a+16)),
        # warming up from seq 16a-K; chain 0 of core 0 starts exactly at 0.
        a0 = 128 * g
        idx = (16 * (a0 + np.arange(P))[:, None] - K
               + np.arange(LEN)[None, :])          # [P, LEN]
        if g == 0:
            idx[0] = np.arange(LEN)
        win = fe_all[idx]                           # [P, LEN, TAG]
        fes = np.ascontiguousarray(win.reshape(P, LEN * TAG))

        x0 = np.ones((TAG, P), np.float32)
        if g == 0:
            x0[:, 0] = 0.0
            x0[start_i, 0] = 1.0
        x0_t = np.ascontiguousarray(
            x0.reshape(NT, P, P).transpose(1, 0, 2).reshape(P, NT * P)
        ).astype(BF)
        in_maps.append({
            "mhat": mhat, "ucol": ucol, "ident": ident, "initx": x0_t,
            "fes": fes,
        })

    if _compiled is None:
        _compiled = _build_kernel()
    res = run_bass_kernel_spmd(_compiled, in_maps, list(range(NCORES)))
    results = res.results

    # ---- stitch (host: ~2k scalars)
    def rec(slot):
        return np.concatenate(
            [sum(results[g]["sums"][4 * slot + i] for i in range(4))
             for g in range(NCORES)]).astype(np.float64)

    recK = rec(0)      # norm at chunk-start boundary (after warm-up)
    recL = rec(1)      # norm at end of chunk 0 (chain 0 of core 0 only)
    end = rec(2)       # norm at chain end
    d = float(results[NCORES - 1]["sums"][12][P - 1])

    fs = (np.log(d) - np.log(end[TAG - 1])
          + float(np.sum(np.log(end[1:]) - np.log(recK[1:])))
          + np.log(recL[0]) + SEQ * DELTA)
    out = (fs - gold_vec).astype(np.float32)
    return out


# revision 26
# speedup vs baseline: 1.5492x; 1.0394x over previous
"""CRF loss kernel for Trainium2 (8 NeuronCores, Bass/Tile).

Math
----
The reference computes, for a single sequence of SEQ=16384 steps over
TAG=1024 tags:

  forward:  fv_{t+1}[j] = logsumexp_i(fv_t[i] + T[j,i]) + feat_t[j]
  score    = logsumexp_j(fv_SEQ[j] + T[stop,j])
  output   = score - gold_score[k]            (gold is a cheap exact term)

In real space with E = exp(T) this is p_{t+1} = exp(feat_t) * (E @ p_t) —
a chain of 16384 matvecs with one fixed positive matrix.  Products of
positive random matrices forget their initial direction extremely fast
(measured ~15x error decay per step, far below working precision after
2 steps), so the chain is split into 1024 chunks of L=16 steps.  Chunk b
is evaluated by an independent chain that starts K=2 steps early
(warm-up) from an arbitrary positive vector; after warm-up its direction
equals the true forward direction to working precision.  The scalar
magnitude is recovered by telescoping per-chunk log-norm ratios, which
only needs each chain's vector 1-norm at its chunk boundary and at its
end.

All 1024 chains run in lockstep: 128 chains per core * 8 cores, each
core doing LEN=18 steps.  One step per core is:

  PSUM q[b=128, j'=1024] = sum_i X[i, b] * Mhat[i, j']   (16 accumulating
        128x128-stationary bf16 matmuls, moving = resident Mhat)
  S = q * FE[s]     (4 DVE quarter-multiplies with preloaded fp8
                     exp(feat) rows, bf16 out)
  X' = S^T          (8 PE transposes into 2 rotating PSUM banks +
                     copies back to SBUF split over DVE and ScalarE)

The matmul datapath is bf16 (fp32 matmul streams at 1/4 rate on trn2;
fp8 DoubleRow measured slower here because the stationary operand
changes every matmul and DoubleRow disables fast-weight-load).  fp32
PSUM accumulation keeps per-step log-increment error ~1e-4 nats, far
inside the telescoping stitch's error budget.

Scheduling: state/q/S live in independent quarter-tiles so dependencies
are tile-precise, and the PE emission interleaves step s's accumulation
groups with step s-1's tail transposes and step s's leading transposes
(disjoint PSUM banks, skip_group_check) — the PE queue never drains, so
the HAM clock-gate stays at full rate.  All inputs that are fixed
functions of the problem (Mhat = exp(T^T-delta) in bf16, exp(feats) in
fp8e4m3 arranged per-chain so no partition-shifted loads are needed,
u = exp(T[stop]), the gold score) are precomputed on the host; every
DMA is a pure prefetch spread over both HWDGE queues and the gpsimd
SWDGE queue, and the 13 result scalars per chain leave through a single
on-device transpose + one contiguous DMA.  delta=8 keeps values
centered (per-step norm growth is ~e^8); drift over 18 steps is a few
e-folds so no per-step normalization is needed.
"""

import sys
import numpy as np
import ml_dtypes

for _p in ("/opt/trn_rl_repo",):
    if _p not in sys.path:
        sys.path.insert(0, _p)

from contextlib import ExitStack

from concourse import bacc, tile
from concourse import mybir
from concourse.bass_utils import run_bass_kernel_spmd

F32 = mybir.dt.float32
BF16 = mybir.dt.bfloat16
FP8 = mybir.dt.float8e4
BF = ml_dtypes.bfloat16
F8 = mybir.dt.np(mybir.dt.float8e4)

SEQ = 16384
TAG = 1024
P = 128            # partitions / chains per core / PE tile edge
NT = TAG // P      # 8 tag tiles
NCORES = 8
L = 16             # chunk length (steps per chunk)
K = 1              # warm-up steps per chain
LEN = L + K        # lockstep steps per core
DELTA = 8.0        # per-step log-growth folded into Mhat
ROWS_PER_CORE = L * P  # 2048

_compiled = None


def _build_kernel():
    nc = bacc.Bacc(
        "TRN2",
        target_bir_lowering=False,
        debug=False,
        num_devices=NCORES,
    )

    mhat_d = nc.declare_dram_parameter("mhat", [P, NT * TAG], BF16,
                                       isOutput=False)
    ident = nc.declare_dram_parameter("ident", [P, P], BF16, isOutput=False)
    ucol_d = nc.declare_dram_parameter("ucol", [P, NT], BF16, isOutput=False)
    initx = nc.declare_dram_parameter("initx", [P, TAG], BF16, isOutput=False)
    fes_d = nc.declare_dram_parameter("fes", [P, LEN * TAG], FP8,
                                      isOutput=False)
    sums = nc.declare_dram_parameter("sums", [16, P], F32, isOutput=True)

    with tile.TileContext(nc) as tc, ExitStack() as ctx:
        const_pool = ctx.enter_context(tc.tile_pool(name="const", bufs=1))
        loop_sb = ctx.enter_context(tc.tile_pool(name="loop_sb", bufs=2))
        qpool = ctx.enter_context(
            tc.tile_pool(name="qpool", bufs=2, space="PSUM"))
        xppool = ctx.enter_context(
            tc.tile_pool(name="xppool", bufs=1, space="PSUM"))

        # state lives in 4 independent quarter-tiles (2 tag-blocks each) so
        # the transpose handoff of quarter i only gates the matmuls reading
        # quarter i — the handoff pipeline overlaps the next MM phase.
        xtq = [loop_sb.tile([P, 2, P], BF16, tag=f"xt{i}", name=f"xt{i}")
               for i in range(4)]
        for i in range(4):
            eng = (nc.sync, nc.scalar)[i % 2]
            eng.dma_start(xtq[i][:], initx[:, 2 * i * P:(2 * i + 2) * P])

        # Mhat resident in SBUF; 8 it-block DMAs over the two HWDGE queues
        # (DMA triggers cost ~600ns of issuing-engine time, so few and big).
        mh = const_pool.tile([P, NT * TAG], BF16)
        for c in range(NT):
            lo = c * TAG
            eng = (nc.sync, nc.scalar)[c % 2]
            eng.dma_start(mh[:, lo:lo + TAG], mhat_d[:, lo:lo + TAG])

        idt = const_pool.tile([P, P], BF16)
        nc.sync.dma_start(idt[:], ident[:])
        idt32 = const_pool.tile([P, P], F32)
        nc.scalar.copy(idt32[:], idt[:])
        ucol = const_pool.tile([P, NT], BF16)
        nc.scalar.dma_start(ucol[:], ucol_d[:])
        recs = const_pool.tile([P, 16], F32)
        nc.gpsimd.memset(recs[:], 0.0)

        # exp(feat) rows (fp8), one window of LEN rows per chain (pre-arranged
        # on host so partition p holds exactly chain p's rows).  Slices for
        # the first 8 steps ride the two HWDGE queues right behind Mhat;
        # the tail goes through the otherwise-idle gpsimd SWDGE queue.
        fes = const_pool.tile([P, LEN * TAG], FP8)
        for c in range(4):
            lo = 2 * c * TAG
            eng = (nc.sync, nc.scalar)[c % 2]
            eng.dma_start(fes[:, lo:lo + 2 * TAG], fes_d[:, lo:lo + 2 * TAG])
        for c in range(3):
            lo = (8 + 4 * c) * TAG
            hi = min(LEN * TAG, lo + 4 * TAG)
            nc.gpsimd.dma_start(fes[:, lo:hi], fes_d[:, lo:hi])

        rec_slot = {K - 1: 0, L - 1: 1, LEN - 1: 2}

        # PE emission interleaves step s's matmuls with step s-1's tail
        # transposes and step s's own leading transposes so the PE queue
        # never drains while the DVE multiply/copy handoff runs.  The
        # transposes land in PSUM banks disjoint from the accumulating
        # q banks, so splicing them inside an accumulation group is safe
        # (skip_group_check).
        def emit_T(jt, stq, xtq, xpab, s):
            xp = xpab[jt // 4][:, (jt % 4) * P:(jt % 4 + 1) * P]
            nc.tensor.matmul(
                xp, lhsT=stq[jt // 2][:, (jt % 2) * P:(jt % 2 + 1) * P],
                rhs=idt[:], is_transpose=True, skip_group_check=True)
            dst = xtq[jt // 2][:, jt % 2, :]
            if jt in (4, 5):
                nc.scalar.copy(dst, xp)
            else:
                nc.vector.tensor_copy(dst, xp)

        def emit_mul(i, qh, stq, s):
            qs = qh[i // 2][:, (i % 2) * 256:(i % 2 + 1) * 256]
            fs = fes[:, s * TAG + i * 256: s * TAG + (i + 1) * 256]
            nc.vector.tensor_mul(stq[i][:], qs, fs)
            if s in rec_slot:
                nc.vector.tensor_reduce(
                    out=recs[:, 4 * rec_slot[s] + i: 4 * rec_slot[s] + i + 1],
                    in_=stq[i][:], op=mybir.AluOpType.add,
                    axis=mybir.AxisListType.X)

        prev = None    # (stq, xtq_new, xpab, s) of previous step
        for s in range(LEN):
            qh = [qpool.tile([P, 512], F32, tag=f"q{h}", name=f"q{h}_{s}")
                  for h in range(2)]
            stq = [loop_sb.tile([P, 256], BF16, tag=f"st{i}",
                                name=f"st{i}_{s}") for i in range(4)]
            nxtq = [loop_sb.tile([P, 2, P], BF16, tag=f"xt{i}",
                                 name=f"xt{i}_{s}") for i in range(4)]
            xpab = (xppool.tile([P, 512], BF16, tag="xpa", name=f"xpa_{s}"),
                    xppool.tile([P, 512], BF16, tag="xpb", name=f"xpb_{s}"))

            for it in range(4):
                nc.tensor.matmul(
                    qh[0][:, :], lhsT=xtq[it // 2][:, it % 2, :],
                    rhs=mh[:, it * TAG: it * TAG + 512],
                    start=(it == 0), stop=False, skip_group_check=True)
            if prev is not None:
                for jt in (4, 5, 6, 7):
                    emit_T(jt, *prev)
            for it in range(4, NT):
                nc.tensor.matmul(
                    qh[0][:, :], lhsT=xtq[it // 2][:, it % 2, :],
                    rhs=mh[:, it * TAG: it * TAG + 512],
                    start=False, stop=(it == NT - 1), skip_group_check=True)
            emit_mul(0, qh, stq, s)
            emit_mul(1, qh, stq, s)
            for it in range(4):
                nc.tensor.matmul(
                    qh[1][:, :], lhsT=xtq[it // 2][:, it % 2, :],
                    rhs=mh[:, it * TAG + 512: (it + 1) * TAG],
                    start=(it == 0), stop=False, skip_group_check=True)
            for jt in (0, 1, 2, 3):
                emit_T(jt, stq, nxtq, xpab, s)
            for it in range(4, NT):
                nc.tensor.matmul(
                    qh[1][:, :], lhsT=xtq[it // 2][:, it % 2, :],
                    rhs=mh[:, it * TAG + 512: (it + 1) * TAG],
                    start=False, stop=(it == NT - 1), skip_group_check=True)
            emit_mul(2, qh, stq, s)
            emit_mul(3, qh, stq, s)

            prev = (stq, nxtq, xpab, s)
            xtq = nxtq
        for jt in (4, 5, 6, 7):
            emit_T(jt, *prev)

        # ---- dots[b] = sum_j u[j] * X_end[j, b]  (X_end = S_end^T)
        dots_ps = xppool.tile([P, 1], F32, tag="dots", bufs=1)
        for it in range(NT):
            nc.tensor.matmul(
                dots_ps[:], lhsT=xtq[it // 2][:, it % 2, :],
                rhs=ucol[:, it:it + 1], start=(it == 0),
                stop=(it == NT - 1))
        nc.vector.tensor_copy(recs[:, 12:13], dots_ps[:])

        # recs [128, 16] -> transpose on PE -> single contiguous DMA out
        rec_ps = xppool.tile([16, P], F32, tag="rec_ps", bufs=1)
        nc.tensor.transpose(rec_ps[:], recs[:], idt32[:])
        rec_sb = const_pool.tile([16, P], F32)
        nc.vector.tensor_copy(rec_sb[:], rec_ps[:])
        nc.sync.dma_start(sums[:], rec_sb[:])

    nc.compile()
    return nc


def kernel(feats, transitions, tags, start_idx, stop_idx):
    global _compiled
    feats = np.ascontiguousarray(np.asarray(feats, dtype=np.float32))
    T = np.ascontiguousarray(np.asarray(transitions, dtype=np.float32))
    tags_np = np.asarray(tags).astype(np.int64)
    start_i = int(np.asarray(start_idx))
    stop_i = int(np.asarray(stop_idx))

    # ---- gold score entirely on host (cheap, exact)
    tags_ext = np.concatenate([np.array([start_i], dtype=np.int64), tags_np])
    trans_sum = T[tags_ext[1:], tags_ext[:-1]].astype(np.float64).sum()
    counts = np.bincount(tags_ext[1:], minlength=TAG).astype(np.float64)
    emit = counts @ feats[:TAG].astype(np.float64)          # [TAG]
    gold_vec = trans_sum + emit + np.float64(T[stop_i, tags_ext[-1]])

    # ---- fixed input transforms on host
    # Mhat[i, j'] = exp(T[j', i] - DELTA), blocked [128, it*1024 + j']
    Mh = np.exp(T.T.astype(np.float32) - np.float32(DELTA))
    mhat = np.ascontiguousarray(
        Mh.reshape(NT, P, TAG).transpose(1, 0, 2).reshape(P, NT * TAG)
    ).astype(BF)
    # u[p, jt] = exp(T[stop, jt*128+p])
    ucol = np.ascontiguousarray(
        np.exp(T[stop_i].astype(np.float32)).reshape(NT, P).T).astype(BF)
    ident = np.eye(P, dtype=np.float32).astype(BF)

    fe_all = np.exp(feats).astype(F8)       # [SEQ, TAG]

    in_maps = []
    for g in range(NCORES):
        # chain b of core g covers global chunk a=128g+b (seq [16a,16a+16)),
        # warming up from seq 16a-K; chain 0 of core 0 starts exactly at 0.
        a0 = 128 * g
        idx = (16 * (a0 + np.arange(P))[:, None] - K
               + np.arange(LEN)[None, :])          # [P, LEN]
        if g == 0:
            idx[0] = np.arange(LEN)
        win = fe_all[idx]                           # [P, LEN, TAG]
        fes = np.ascontiguousarray(win.reshape(P, LEN * TAG))

        x0 = np.ones((TAG, P), np.float32)
        if g == 0:
            x0[:, 0] = 0.0
            x0[start_i, 0] = 1.0
        x0_t = np.ascontiguousarray(
            x0.reshape(NT, P, P).transpose(1, 0, 2).reshape(P, NT * P)
        ).astype(BF)
        in_maps.append({
            "mhat": mhat, "ucol": ucol, "ident": ident, "initx": x0_t,
            "fes": fes,
        })

    if _compiled is None:
        _compiled = _build_kernel()
    res = run_bass_kernel_spmd(_compiled, in_maps, list(range(NCORES)))
    results = res.results

    # ---- stitch (host: ~2k scalars)
    def rec(slot):
        return np.concatenate(
            [sum(results[g]["sums"][4 * slot + i] for i in range(4))
             for g in range(NCORES)]).astype(np.float64)

    recK = rec(0)      # norm at chunk-start boundary (after warm-up)
    recL = rec(1)      # norm at end of chunk 0 (chain 0 of core 0 only)
    end = rec(2)       # norm at chain end
    d = float(results[NCORES - 1]["sums"][12][P - 1])

    fs = (np.log(d) - np.log(end[TAG - 1])
          + float(np.sum(np.log(end[1:]) - np.log(recK[1:])))
          + np.log(recL[0]) + SEQ * DELTA)
    out = (fs - gold_vec).astype(np.float32)
    return out
